# revision 38
# baseline (speedup 1.0000x reference)
"""Per-pixel adaptive 5x5 conv (KPN) for Trainium2, 8-core data parallel.

out[g,h,w] = sum_{i,j} core[g,5i+j,h,w] * frames_pad[g,h+i-2,w+j-2]
with g = flattened (B,N) = 16 image planes; 2 planes per NeuronCore,
fused into one free dim (FD=4096) so every elementwise op covers both.

Engine split (GpSimd stays idle: it shares a physical SBUF port with
VectorE and concurrent streaming slows DVE tensor_tensor ~4.5x):
  DVE    - 25 products w_t*f_t (fp16 2x mode, 2.3us each); also
           dequantizes 5 weight tiles (tensor_copy int8->fp16)
  ACT    - dequantizes the other 20 weight tiles (activation-copy
           int8->fp16); evacuates PSUM fp32 -> SBUF fp16 at the end
  PE     - accumulates the 25 product streams into PSUM (fp32) via
           matmuls against a stationary (2^-5 * I); 8 banks = [128,4096]
  sync   - all DMAs, emitted in consume order

The DMA system is SBUF-WRITE-side limited (~27 GB/s per queue x 16;
measured: a casting DMA costs the same as its fp16 write size), so
weights are stored int8 (w8 = clip(round(w * 32), -127, 127)) and
DMA'd as int8 (queue writes 26.2 -> 13.1 MB/core) then cast to fp16
on chip by the otherwise-idle ACT engine (plus 5 tiles on DVE slack;
k=2 of each group is the only arrival-safe slot - the DMA stream runs
only ~1 group ahead of compute, and later-k dequants stall DVE).
The 2^-5 dequant scale is folded into the PE's stationary identity,
so dequant is a pure cast.  Measured end-to-end rel err 9.4e-3
(gate 2e-2).

Host layouts:
  fin [5, 2, 128, 4144] fp16: fin[i, par, p, (img,blk,c)] =
     Fpad[img, blk*128+p+i, (1-par)+c], c in [0,518).  Parity copies
     keep every tap's 512-col slice 4-byte aligned for DVE 2x mode.
  win [25, 128, 4096] int8: win[t, p, (img,blk,c)] =
     clip(round(32*core[img, t, blk*128+p, c]), -127, 127)
  oout [128, 4096] fp16 (host casts to f32).
"""

import os
import sys

import numpy as np

for _p in ("/opt/trn_rl_repo",):
    if _p not in sys.path and os.path.isdir(_p):
        sys.path.insert(0, _p)

K = 5
NCORES = 8
IMGS_PER_CORE = 2
H = W = 512
NBLK = 4  # 128-row blocks per image
FCOLS = 518
F_FREE = IMGS_PER_CORE * NBLK * FCOLS  # 4144 per parity tile
W_FREE = IMGS_PER_CORE * NBLK * W  # 4096
O_FREE = IMGS_PER_CORE * NBLK * W  # 4096
NBANK = 8
BANK = O_FREE // NBANK  # 512 fp32 per PSUM bank
WSCALE = 2.0 ** -5  # int8 weight dequant scale, folded into PE identity

# All 25 product streams go straight to PE (it has headroom at ~2.7us
# per stream); weight dequant splits DVE 5 / ACT 20 so both engines
# land at ~72us: DVE 25 muls (57us) + 5 casts + PSUM evac, ACT 20
# casts (~3.6us each).
PAIR_K = ()
DVE_DEQ = frozenset({2, 7, 12, 17, 22})
SWDGE_W = frozenset()

_compiled = {}
last_results = None  # BassKernelResults of the most recent run (for test.py)


def _build_nc():
    import concourse.bacc as bacc
    import concourse.mybir as mybir
    from concourse.tile import TileContext

    f16 = mybir.dt.float16
    f32 = mybir.dt.float32
    i8 = mybir.dt.int8

    nc = bacc.Bacc(None, target_bir_lowering=False, debug=False)
    ident = nc.dram_tensor("ident", [128, 128], f16, kind="ExternalInput")
    fin = nc.dram_tensor("fin", [K, 2, 128, F_FREE], f16,
                         kind="ExternalInput")
    win = nc.dram_tensor("win", [K * K, 128, W_FREE], i8,
                         kind="ExternalInput")
    oout = nc.dram_tensor("oout", [128, O_FREE], f16,
                          kind="ExternalOutput")

    n_streams = K * K

    with TileContext(nc) as tc:
        with (
            tc.tile_pool(name="ipool", bufs=1) as ipool,
            tc.tile_pool(name="fpool", bufs=2) as fpool,
            tc.tile_pool(name="w8pool", bufs=2) as w8pool,
            tc.tile_pool(name="wpool", bufs=2) as wpool,
            tc.tile_pool(name="spool", bufs=4) as spool,
            tc.tile_pool(name="opool", bufs=1) as opool,
            tc.tile_pool(name="ppool", bufs=1, space="PSUM") as ppool,
        ):
            id_t = ipool.tile([128, 128], f16, tag="ident")
            nc.sync.dma_start(out=id_t[:], in_=ident[:])

            banks = [ppool.tile([128, BANK], f32, tag=f"b{b}",
                                name=f"bank{b}")
                     for b in range(NBANK)]
            osb = opool.tile([128, O_FREE], f16, tag="osb")

            f_tiles = {}
            w8_tiles = {}
            w_tiles = {}
            stream_idx = [0]

            def emit_w(tg, k):
                t = tg * K + k
                if t in SWDGE_W:
                    # casting DMA writes dequantized fp16 directly
                    w_t = wpool.tile([128, W_FREE], f16, tag=f"w{k}",
                                     name=f"wsw{t}")
                    nc.gpsimd.dma_start(out=w_t[:], in_=win[t])
                    w_tiles[t] = w_t
                    return
                w8_t = w8pool.tile([128, W_FREE], i8, tag=f"w8{k}",
                                   name=f"w8_{t}")
                nc.sync.dma_start(out=w8_t[:], in_=win[t])
                w8_tiles[t] = w8_t
                # ACT dequants (int8 codes -> fp16) are emitted here so
                # ACT runs a group ahead of the DVE muls; DVE's own
                # dequants are emitted inline in emit_compute.
                if t not in DVE_DEQ:
                    w_t = wpool.tile([128, W_FREE], f16, tag=f"w{k}",
                                     name=f"wdq{t}")
                    nc.scalar.copy(out=w_t[:], in_=w8_t[:])
                    w_tiles[t] = w_t

            def emit_dmas(tg):
                # w8(k0) first: ACT's dequant chain is the critical path
                # and its first op needs only that 0.5MB tile, not the
                # 1MB frame tile (which arrives in parallel for the mul).
                emit_w(tg, 0)
                f_t = fpool.tile([128, F_FREE], f16, tag="f0",
                                 name=f"fr{tg}p0")
                nc.sync.dma_start(out=f_t[:], in_=fin[tg, 0])
                f_tiles[(tg, 0)] = f_t
                emit_w(tg, 1)
                f_t = fpool.tile([128, F_FREE], f16, tag="f1",
                                 name=f"fr{tg}p1")
                nc.sync.dma_start(out=f_t[:], in_=fin[tg, 1])
                f_tiles[(tg, 1)] = f_t
                for k in range(2, K):
                    emit_w(tg, k)

            def pe_accumulate(tile):
                s = stream_idx[0]
                stream_idx[0] += 1
                for b in range(NBANK):
                    nc.tensor.matmul(
                        out=banks[b][:],
                        lhsT=id_t[:],
                        rhs=tile[:][:, b * BANK:(b + 1) * BANK],
                        start=(s == 0),
                        stop=(s == n_streams - 1),
                    )

            def emit_compute(tg):
                pair_tile = None
                for k in range(K):
                    t = tg * K + k
                    if t in DVE_DEQ:
                        w_t = wpool.tile([128, W_FREE], f16, tag=f"w{k}")
                        nc.vector.tensor_copy(out=w_t[:],
                                              in_=w8_tiles[t][:])
                        w_tiles[t] = w_t
                    j = k
                    par = j & 1
                    joff = j + par
                    fv = f_tiles[(tg, par)][:].rearrange(
                        "p (img blk c) -> p img blk c",
                        img=IMGS_PER_CORE, blk=NBLK, c=FCOLS)
                    f_ap = fv[:, :, :, joff:joff + W]
                    w_ap = w_tiles[t][:].rearrange(
                        "p (img blk c) -> p img blk c",
                        img=IMGS_PER_CORE, blk=NBLK, c=W)
                    tmp = spool.tile([128, O_FREE], f16, tag="s")
                    tv = tmp[:].rearrange("p (img blk c) -> p img blk c",
                                          img=IMGS_PER_CORE, blk=NBLK, c=W)
                    nc.vector.tensor_mul(out=tv, in0=w_ap, in1=f_ap)
                    if PAIR_K and k == PAIR_K[0]:
                        pair_tile = tmp
                    elif PAIR_K and k == PAIR_K[1]:
                        nc.vector.tensor_add(out=pair_tile[:],
                                             in0=pair_tile[:], in1=tmp[:])
                        pe_accumulate(pair_tile)
                    else:
                        pe_accumulate(tmp)

            emit_dmas(0)
            for tg in range(1, K):
                emit_dmas(tg)
                emit_compute(tg - 1)
            emit_compute(K - 1)

            # PSUM fp32 -> SBUF fp16 per bank on the (otherwise idle)
            # scalar engine, then store halves as they complete.
            for b in range(NBANK):
                nc.scalar.copy(out=osb[:][:, b * BANK:(b + 1) * BANK],
                               in_=banks[b][:])
                if b == NBANK // 2 - 1:
                    nc.sync.dma_start(out=oout[:, :O_FREE // 2],
                                      in_=osb[:][:, :O_FREE // 2])
            nc.sync.dma_start(out=oout[:, O_FREE // 2:],
                              in_=osb[:][:, O_FREE // 2:])
    nc.finalize()
    return nc


def _host_prep(frames, core):
    """Build per-core in_maps. frames [4,4,1,512,512] f32, core [4,4,25,1,512,512]."""
    G = NCORES * IMGS_PER_CORE  # 16
    F = np.ascontiguousarray(frames.reshape(G, H, W))
    Wc = core.reshape(G, K * K, H, W)

    # frames: pad rows 2/2, cols 3/4; Fp[g, r, c] = F[g, r-2, c-3]
    Fp = np.pad(F, ((0, 0), (2, 2), (3, 4))).astype(np.float16)
    # A[g, i, par, blk, p, c] = Fp[g, blk*128+p+i, (1-par)+c]
    A = np.empty((G, K, 2, NBLK, 128, FCOLS), np.float16)
    for i in range(K):
        for par in range(2):
            sl = Fp[:, i:i + H, (1 - par):(1 - par) + FCOLS]  # [G,512,518]
            A[:, i, par] = sl.reshape(G, NBLK, 128, FCOLS)
    # fin[core][i, par, p, (img, blk, c)]
    fprep = np.ascontiguousarray(
        A.reshape(NCORES, IMGS_PER_CORE, K, 2, NBLK, 128, FCOLS)
        .transpose(0, 2, 3, 5, 1, 4, 6))

    # win[core][t, p, (img, blk, c)] as int8 codes of w/2^-5
    w8 = np.clip(np.round(Wc.astype(np.float64) / WSCALE), -127, 127)
    w8 = w8.astype(np.int8).reshape(G, K * K, NBLK, 128, W)
    wprep = np.ascontiguousarray(
        w8.reshape(NCORES, IMGS_PER_CORE, K * K, NBLK, 128, W)
        .transpose(0, 2, 4, 1, 3, 5))

    ident = (np.eye(128) * WSCALE).astype(np.float16)
    in_maps = []
    for c in range(NCORES):
        in_maps.append({
            "ident": ident,
            "fin": fprep[c].reshape(K, 2, 128, F_FREE),
            "win": wprep[c].reshape(K * K, 128, W_FREE),
        })
    return in_maps


def kernel(frames, core, bias):
    global last_results
    from concourse.bass_utils import run_bass_kernel_spmd

    frames = np.asarray(frames, dtype=np.float32)
    core = np.asarray(core, dtype=np.float32)

    if "nc" not in _compiled:
        _compiled["nc"] = _build_nc()
    nc = _compiled["nc"]

    in_maps = _host_prep(frames, core)
    trace = os.environ.get("KC_TRACE") == "1"
    tmpdir = os.environ.get("KC_TRACE_DIR") or None
    if tmpdir:
        os.makedirs(tmpdir, exist_ok=True)
    res = run_bass_kernel_spmd(nc, in_maps, list(range(NCORES)), trace=trace,
                               tmpdir=tmpdir)
    last_results = res

    G = NCORES * IMGS_PER_CORE
    out = np.empty((G, H, W), np.float32)
    for c in range(NCORES):
        o = res.results[c]["oout"]  # [128, 4096] f16
        ov = o.reshape(128, IMGS_PER_CORE, NBLK, W).astype(np.float32)
        for img in range(IMGS_PER_CORE):
            out[c * IMGS_PER_CORE + img] = (
                ov[:, img].transpose(1, 0, 2).reshape(H, W))
    return out.reshape(4, 4, H, W)


# revision 39
# speedup vs baseline: 1.0067x; 1.0067x over previous
"""Per-pixel adaptive 5x5 conv (KPN) for Trainium2, 8-core data parallel.

out[g,h,w] = sum_{i,j} core[g,5i+j,h,w] * frames_pad[g,h+i-2,w+j-2]
with g = flattened (B,N) = 16 image planes; 2 planes per NeuronCore,
fused into one free dim (FD=4096) so every elementwise op covers both.

Engine split (GpSimd stays idle: it shares a physical SBUF port with
VectorE and concurrent streaming slows DVE tensor_tensor ~4.5x):
  DVE    - 25 products w_t*f_t (fp16 2x mode, 2.3us each); also
           dequantizes 5 weight tiles (tensor_copy int8->fp16)
  ACT    - dequantizes the other 20 weight tiles (activation-copy
           int8->fp16); evacuates PSUM fp32 -> SBUF fp16 at the end
  PE     - accumulates the 25 product streams into PSUM (fp32) via
           matmuls against a stationary (2^-5 * I); 8 banks = [128,4096]
  sync   - all DMAs, emitted in consume order

The DMA system is SBUF-WRITE-side limited (~27 GB/s per queue x 16;
measured: a casting DMA costs the same as its fp16 write size), so
weights are stored int8 (w8 = clip(round(w * 32), -127, 127)) and
DMA'd as int8 (queue writes 26.2 -> 13.1 MB/core) then cast to fp16
on chip by the otherwise-idle ACT engine (plus 5 tiles on DVE slack;
k=2 of each group is the only arrival-safe slot - the DMA stream runs
only ~1 group ahead of compute, and later-k dequants stall DVE).
The 2^-5 dequant scale is folded into the PE's stationary identity,
so dequant is a pure cast.  Measured end-to-end rel err 9.4e-3
(gate 2e-2).

Host layouts:
  fin [5, 2, 128, 4144] fp16: fin[i, par, p, (img,blk,c)] =
     Fpad[img, blk*128+p+i, (1-par)+c], c in [0,518).  Parity copies
     keep every tap's 512-col slice 4-byte aligned for DVE 2x mode.
  win [25, 128, 4096] int8: win[t, p, (img,blk,c)] =
     clip(round(32*core[img, t, blk*128+p, c]), -127, 127)
  oout [128, 4096] fp16 (host casts to f32).
"""

import os
import sys

import numpy as np

for _p in ("/opt/trn_rl_repo",):
    if _p not in sys.path and os.path.isdir(_p):
        sys.path.insert(0, _p)

K = 5
NCORES = 8
IMGS_PER_CORE = 2
H = W = 512
NBLK = 4  # 128-row blocks per image
FCOLS = 518
F_FREE = IMGS_PER_CORE * NBLK * FCOLS  # 4144 per parity tile
W_FREE = IMGS_PER_CORE * NBLK * W  # 4096
O_FREE = IMGS_PER_CORE * NBLK * W  # 4096
NBANK = 8
BANK = O_FREE // NBANK  # 512 fp32 per PSUM bank
WSCALE = 2.0 ** -5  # int8 weight dequant scale, folded into PE identity

# All 25 product streams go straight to PE (it has headroom at ~2.7us
# per stream); weight dequant splits DVE 5 / ACT 20 so both engines
# land at ~72us: DVE 25 muls (57us) + 5 casts + PSUM evac, ACT 20
# casts (~3.6us each).
PAIR_K = ()
DVE_DEQ = frozenset({2, 7, 12, 17, 22})
SWDGE_W = frozenset()

_compiled = {}
last_results = None  # BassKernelResults of the most recent run (for test.py)


def _build_nc():
    import concourse.bacc as bacc
    import concourse.mybir as mybir
    from concourse.tile import TileContext

    f16 = mybir.dt.float16
    f32 = mybir.dt.float32
    i8 = mybir.dt.int8

    nc = bacc.Bacc(None, target_bir_lowering=False, debug=False)
    ident = nc.dram_tensor("ident", [128, 128], f16, kind="ExternalInput")
    fin = nc.dram_tensor("fin", [K, 2, 128, F_FREE], f16,
                         kind="ExternalInput")
    win = nc.dram_tensor("win", [K * K, 128, W_FREE], i8,
                         kind="ExternalInput")
    oout = nc.dram_tensor("oout", [128, O_FREE], f16,
                          kind="ExternalOutput")

    n_streams = K * K

    with TileContext(nc) as tc:
        with (
            tc.tile_pool(name="ipool", bufs=1) as ipool,
            tc.tile_pool(name="fpool", bufs=2) as fpool,
            tc.tile_pool(name="w8pool", bufs=2) as w8pool,
            tc.tile_pool(name="wpool", bufs=2) as wpool,
            tc.tile_pool(name="spool", bufs=4) as spool,
            tc.tile_pool(name="opool", bufs=1) as opool,
            tc.tile_pool(name="ppool", bufs=1, space="PSUM") as ppool,
        ):
            id_t = ipool.tile([128, 128], f16, tag="ident")
            nc.sync.dma_start(out=id_t[:], in_=ident[:])

            banks = [ppool.tile([128, BANK], f32, tag=f"b{b}",
                                name=f"bank{b}")
                     for b in range(NBANK)]
            osb = opool.tile([128, O_FREE], f16, tag="osb")

            f_tiles = {}
            w8_tiles = {}
            w_tiles = {}
            stream_idx = [0]

            def emit_w(tg, k):
                t = tg * K + k
                if t in SWDGE_W:
                    # casting DMA writes dequantized fp16 directly
                    w_t = wpool.tile([128, W_FREE], f16, tag=f"w{k}",
                                     name=f"wsw{t}")
                    nc.gpsimd.dma_start(out=w_t[:], in_=win[t])
                    w_tiles[t] = w_t
                    return
                w8_t = w8pool.tile([128, W_FREE], i8, tag=f"w8{k}",
                                   name=f"w8_{t}")
                nc.sync.dma_start(out=w8_t[:], in_=win[t])
                w8_tiles[t] = w8_t
                # ACT dequants (int8 codes -> fp16) are emitted here so
                # ACT runs a group ahead of the DVE muls; DVE's own
                # dequants are emitted inline in emit_compute.
                if t not in DVE_DEQ:
                    w_t = wpool.tile([128, W_FREE], f16, tag=f"w{k}",
                                     name=f"wdq{t}")
                    nc.scalar.copy(out=w_t[:], in_=w8_t[:])
                    w_tiles[t] = w_t

            def emit_dmas(tg):
                # first group: tap-0 path (f par0, w0) ahead of f par1
                f_t = fpool.tile([128, F_FREE], f16, tag="f0",
                                 name=f"fr{tg}p0")
                nc.sync.dma_start(out=f_t[:], in_=fin[tg, 0])
                f_tiles[(tg, 0)] = f_t
                emit_w(tg, 0)
                f_t = fpool.tile([128, F_FREE], f16, tag="f1",
                                 name=f"fr{tg}p1")
                nc.sync.dma_start(out=f_t[:], in_=fin[tg, 1])
                f_tiles[(tg, 1)] = f_t
                for k in range(1, K):
                    emit_w(tg, k)

            def pe_accumulate(tile):
                s = stream_idx[0]
                stream_idx[0] += 1
                for b in range(NBANK):
                    nc.tensor.matmul(
                        out=banks[b][:],
                        lhsT=id_t[:],
                        rhs=tile[:][:, b * BANK:(b + 1) * BANK],
                        start=(s == 0),
                        stop=(s == n_streams - 1),
                    )

            def emit_compute(tg):
                pair_tile = None
                for k in range(K):
                    t = tg * K + k
                    if t in DVE_DEQ:
                        w_t = wpool.tile([128, W_FREE], f16, tag=f"w{k}")
                        nc.vector.tensor_copy(out=w_t[:],
                                              in_=w8_tiles[t][:])
                        w_tiles[t] = w_t
                    j = k
                    par = j & 1
                    joff = j + par
                    fv = f_tiles[(tg, par)][:].rearrange(
                        "p (img blk c) -> p img blk c",
                        img=IMGS_PER_CORE, blk=NBLK, c=FCOLS)
                    f_ap = fv[:, :, :, joff:joff + W]
                    w_ap = w_tiles[t][:].rearrange(
                        "p (img blk c) -> p img blk c",
                        img=IMGS_PER_CORE, blk=NBLK, c=W)
                    tmp = spool.tile([128, O_FREE], f16, tag="s")
                    tv = tmp[:].rearrange("p (img blk c) -> p img blk c",
                                          img=IMGS_PER_CORE, blk=NBLK, c=W)
                    nc.vector.tensor_mul(out=tv, in0=w_ap, in1=f_ap)
                    if PAIR_K and k == PAIR_K[0]:
                        pair_tile = tmp
                    elif PAIR_K and k == PAIR_K[1]:
                        nc.vector.tensor_add(out=pair_tile[:],
                                             in0=pair_tile[:], in1=tmp[:])
                        pe_accumulate(pair_tile)
                    else:
                        pe_accumulate(tmp)

            emit_dmas(0)
            for tg in range(1, K):
                emit_dmas(tg)
                emit_compute(tg - 1)
            emit_compute(K - 1)

            # PSUM fp32 -> SBUF fp16 per bank on the (otherwise idle)
            # scalar engine, then store halves as they complete.
            for b in range(NBANK):
                nc.scalar.copy(out=osb[:][:, b * BANK:(b + 1) * BANK],
                               in_=banks[b][:])
                if b == NBANK // 2 - 1:
                    nc.sync.dma_start(out=oout[:, :O_FREE // 2],
                                      in_=osb[:][:, :O_FREE // 2])
            nc.sync.dma_start(out=oout[:, O_FREE // 2:],
                              in_=osb[:][:, O_FREE // 2:])
    nc.finalize()
    return nc


def _host_prep(frames, core):
    """Build per-core in_maps. frames [4,4,1,512,512] f32, core [4,4,25,1,512,512]."""
    G = NCORES * IMGS_PER_CORE  # 16
    F = np.ascontiguousarray(frames.reshape(G, H, W))
    Wc = core.reshape(G, K * K, H, W)

    # frames: pad rows 2/2, cols 3/4; Fp[g, r, c] = F[g, r-2, c-3]
    Fp = np.pad(F, ((0, 0), (2, 2), (3, 4))).astype(np.float16)
    # A[g, i, par, blk, p, c] = Fp[g, blk*128+p+i, (1-par)+c]
    A = np.empty((G, K, 2, NBLK, 128, FCOLS), np.float16)
    for i in range(K):
        for par in range(2):
            sl = Fp[:, i:i + H, (1 - par):(1 - par) + FCOLS]  # [G,512,518]
            A[:, i, par] = sl.reshape(G, NBLK, 128, FCOLS)
    # fin[core][i, par, p, (img, blk, c)]
    fprep = np.ascontiguousarray(
        A.reshape(NCORES, IMGS_PER_CORE, K, 2, NBLK, 128, FCOLS)
        .transpose(0, 2, 3, 5, 1, 4, 6))

    # win[core][t, p, (img, blk, c)] as int8 codes of w/2^-5
    w8 = np.clip(np.round(Wc.astype(np.float64) / WSCALE), -127, 127)
    w8 = w8.astype(np.int8).reshape(G, K * K, NBLK, 128, W)
    wprep = np.ascontiguousarray(
        w8.reshape(NCORES, IMGS_PER_CORE, K * K, NBLK, 128, W)
        .transpose(0, 2, 4, 1, 3, 5))

    ident = (np.eye(128) * WSCALE).astype(np.float16)
    in_maps = []
    for c in range(NCORES):
        in_maps.append({
            "ident": ident,
            "fin": fprep[c].reshape(K, 2, 128, F_FREE),
            "win": wprep[c].reshape(K * K, 128, W_FREE),
        })
    return in_maps


def kernel(frames, core, bias):
    global last_results
    from concourse.bass_utils import run_bass_kernel_spmd

    frames = np.asarray(frames, dtype=np.float32)
    core = np.asarray(core, dtype=np.float32)

    if "nc" not in _compiled:
        _compiled["nc"] = _build_nc()
    nc = _compiled["nc"]

    in_maps = _host_prep(frames, core)
    trace = os.environ.get("KC_TRACE") == "1"
    tmpdir = os.environ.get("KC_TRACE_DIR") or None
    if tmpdir:
        os.makedirs(tmpdir, exist_ok=True)
    res = run_bass_kernel_spmd(nc, in_maps, list(range(NCORES)), trace=trace,
                               tmpdir=tmpdir)
    last_results = res

    G = NCORES * IMGS_PER_CORE
    out = np.empty((G, H, W), np.float32)
    for c in range(NCORES):
        o = res.results[c]["oout"]  # [128, 4096] f16
        ov = o.reshape(128, IMGS_PER_CORE, NBLK, W).astype(np.float32)
        for img in range(IMGS_PER_CORE):
            out[c * IMGS_PER_CORE + img] = (
                ov[:, img].transpose(1, 0, 2).reshape(H, W))
    return out.reshape(4, 4, H, W)
